# revision 1
# baseline (speedup 1.0000x reference)
"""Distributed multi-head attention kernel for 8 TRN2 NeuronCores.

Reference computation (per batch b):
    q = x @ wq.T ; k = x @ wk.T ; v = x @ wv.T          (heads split from 512 -> 8 x 64)
    attn = softmax(q k^T / sqrt(64)) ; o = attn @ v
    y = concat_heads(o) @ wproj.T

Sharding: core c handles batch b = c // 4 and head-block hb = c % 4
(2 heads = 128 channels).  Each core computes its 128-channel PARTIAL
of the full output projection, and ReduceScatter(add) over the 4-core
replica group both sums the partials and scatters query-column
quarters.  Three RS parts (after query chunks 2 / 4 / 6), in bf16,
overlap the collective with the remaining attention compute.

Engine choreography (the reason for most of the structure below):
  - PE (tensor): qk, av, projection, and the softmax-denominator
    broadcast (ones-column matmul) - the Pool engine must stay free
    because an in-flight collective parks the Pool queue.
  - Scalar (Activation): native Exp for head 0 of every key tile.
  - Vector (DVE): Schraudolph fast exp for head 1 (one fused
    multiply-add into an int16 view of the bf16 output: the integer
    bits of (x*184.665 + 16256.5) ARE the bf16 pattern of ~2^x*log2e),
    plus reciprocals / normalize multiplies.
  - av(mi) is issued two key-tiles behind qk(mi) so the in-order PE
    queue never parks waiting for an exp result.

All device matmuls run in bf16 with fp32 PSUM accumulation.
"""

import sys

sys.path.insert(0, "/opt/trn_rl_repo")

import ml_dtypes
import numpy as np

B = 2
N = 3136
DIM = 512
HEADS = 8
HD = 64
SCALE = HD**-0.5
N_CORES = 8
GROUPS = [[0, 1, 2, 3], [4, 5, 6, 7]]

BF16 = ml_dtypes.bfloat16

# query/row chunks of 512 (last 64), key tiles of 128 (last 64)
QCH = [(o, min(512, N - o)) for o in range(0, N, 512)]
MT = [(o, min(128, N - o)) for o in range(0, N, 128)]

# ReduceScatter parts: (fire after chunk, col start, col width).
# Within each part, replica-rank r owns the r-th quarter of the columns.
PARTS = [(2, 0, 1536), (4, 1536, 1024), (6, 2560, 576)]
OUT_W = sum(w // 4 for _, _, w in PARTS)  # 784 output cols per core

# Schraudolph fast-exp constants for bf16 output via int16 bits:
# int16(x * 128/ln2 + (127*128 + 0.5)) , bit pattern read as bf16 ~= e^x
EXP_A = 184.66497
EXP_B = 16256.5

_CACHE = {}


def _build(debug_dumps=False):
    import concourse.bacc as bacc
    import concourse.masks as masks
    import concourse.mybir as mybir
    import concourse.tile as tile
    from concourse.bass_interp import get_hw_module

    F32 = mybir.dt.float32
    F32R = mybir.dt.float32r
    BF = mybir.dt.bfloat16
    I16 = mybir.dt.int16

    nc = bacc.Bacc("TRN2", target_bir_lowering=False, debug=False, num_devices=N_CORES)

    xT_d = nc.dram_tensor("xT", [DIM, N], BF, kind="ExternalInput")
    wq_d = nc.dram_tensor("wqT", [DIM, 128], BF, kind="ExternalInput")
    wk_d = nc.dram_tensor("wkT", [DIM, 128], BF, kind="ExternalInput")
    wv_d = nc.dram_tensor("wvT", [DIM, 128], BF, kind="ExternalInput")
    wp_d = nc.dram_tensor("wpT", [128, DIM], BF, kind="ExternalInput")
    out_d = nc.dram_tensor("out", [DIM, OUT_W], BF, kind="ExternalOutput")

    EXP = mybir.ActivationFunctionType.Exp
    COPY = mybir.ActivationFunctionType.Copy
    MULT = mybir.AluOpType.mult
    ADD = mybir.AluOpType.add

    with tile.TileContext(nc) as tc:
        with (
            tc.tile_pool(name="const", bufs=1) as cp,
            tc.tile_pool(name="big", bufs=1) as bp,
            tc.tile_pool(name="attn", bufs=4) as ap_,
            tc.tile_pool(name="norm", bufs=3) as np_,
            tc.tile_pool(name="psum", bufs=4, space="PSUM") as pa,
            tc.tile_pool(name="psum1", bufs=2, space="PSUM") as pb,
            tc.tile_pool(name="proj", bufs=2) as pj,
            tc.tile_pool(name="dram", bufs=1, space="DRAM") as dram,
        ):
            # ---- load inputs (weights first: they gate the first matmuls;
            # issue them BEFORE the warmup collective, which parks the Pool
            # queue for its full duration) ----
            wqT = cp.tile([128, 4, 128], BF)
            wkT = cp.tile([128, 4, 128], BF)
            wvT = cp.tile([128, 4, 128], BF)
            wpT = cp.tile([64, 2, DIM], BF)  # [chan%64, head, out_chan]
            idn = cp.tile([128, 128], BF)
            masks.make_identity(nc, idn[:])
            for t, d in ((wkT, wk_d), (wqT, wq_d), (wvT, wv_d)):
                for k in range(4):
                    nc.gpsimd.dma_start(t[:, k, :], d[128 * k : 128 * (k + 1), :])
            for h in range(2):
                nc.gpsimd.dma_start(wpT[:, h, :], wp_d[64 * h : 64 * (h + 1), :])
            xT = bp.tile([128, 4, N], BF)  # xT[:, k, :] = channels [128k,128k+128)
            for lo, hi in ((0, 512), (512, 1792), (1792, N)):
                for k in range(4):
                    nc.sync.dma_start(
                        xT[:, k, lo:hi], xT_d[128 * k : 128 * (k + 1), lo:hi]
                    )

            # ---- tiny warmup collective: absorbs collective-subsystem init
            # concurrently with the compute prologue ----
            wtiny = cp.tile([4, 16], F32)
            nc.vector.memset(wtiny[:], 0.0)
            wi = dram.tile([4, 16], F32)
            wo = dram.tile([1, 16], F32)
            nc.gpsimd.dma_start(wi[:], wtiny[:])
            nc.gpsimd.collective_compute(
                "ReduceScatter",
                ADD,
                replica_groups=GROUPS,
                ins=[wi.opt()],
                outs=[wo.opt()],
            )

            # ---- qkv projections ----
            qT = bp.tile([128, N], BF)  # rows 0-63 head0, 64-127 head1
            kT = bp.tile([128, N], BF)
            v1 = bp.tile([128, len(MT), 2, HD + 1], BF)  # [key, mtile, head, hd|1]
            nc.vector.memset(v1[:, :, :, HD : HD + 1], 1.0)

            aux_n = [0]

            def aux_copy(dst, src):
                # alternate the two elementwise engines for PSUM->SBUF copies
                # (Copy shares the Exp activation table set - no reload cost)
                if aux_n[0] % 2 == 0:
                    nc.scalar.activation(dst, src, COPY)
                else:
                    nc.vector.tensor_copy(dst, src)
                aux_n[0] += 1

            def produce_chunk(wt, dst, qo, qn):
                ps = pa.tile([128, 512], F32, tag="ps", name="ps")
                for k in range(4):
                    nc.tensor.matmul(
                        ps[:, :qn],
                        wt[:, k, :],
                        xT[:, k, qo : qo + qn],
                        start=(k == 0),
                        stop=(k == 3),
                    )
                aux_copy(dst[:, qo : qo + qn], ps[:, :qn])

            def produce_v1(mi):
                mo, mn = MT[mi]
                ps = pa.tile([128, 512], F32, tag="ps", name="ps")
                for k in range(4):
                    nc.tensor.matmul(
                        ps[:mn, :128],
                        xT[:, k, mo : mo + mn],
                        wvT[:, k, :],
                        start=(k == 0),
                        stop=(k == 3),
                    )
                aux_copy(v1[:mn, mi, 0, 0:HD], ps[:mn, 0:HD])
                aux_copy(v1[:mn, mi, 1, 0:HD], ps[:mn, HD:128])

            # up-front: just enough to start chunk 0 (kT for the first two
            # key chunks, qT chunk 0, first v tiles); the rest is produced
            # just-in-time inside the first chunks' key loops so the exp
            # engines start as early as possible.
            for qo, qn in QCH[0:2]:
                produce_chunk(wkT, kT, qo, qn)
            for qo, qn in QCH[0:2]:
                produce_chunk(wqT, qT, qo, qn)
            for mi in range(2):
                produce_v1(mi)

            # ---- attention ----
            outn = [bp.tile([64, N], BF, name=f"outn{h}") for h in range(2)]
            # av-output in [query-partition, hd] layout, 25 qtile slots
            outq = [bp.tile([128, 25, HD], BF, name=f"outq{h}") for h in range(2)]
            # ReduceScatter staging (block r = rows [512r, 512(r+1)) = rank-r quarter)
            ri = [
                dram.tile([4 * DIM, w // 4], BF, name=f"ri{k}")
                for k, (_, _, w) in enumerate(PARTS)
            ]
            ro = [
                dram.tile([DIM, w // 4], BF, name=f"ro{k}")
                for k, (_, _, w) in enumerate(PARTS)
            ]

            def fast_exp(at_ap, pp_ap):
                # DVE Schraudolph: int16 bits of (x*A + B) viewed as bf16
                nc.vector.tensor_scalar(
                    at_ap.bitcast(I16), pp_ap, EXP_A, EXP_B, MULT, ADD
                )

            def normalize(po, qo, qn):
                # softmax denominators live at po[..., 64], one per query
                # PARTITION - so normalize is a per-partition tensor_scalar
                # multiply, then a PE transpose turns each [q,hd] tile back
                # into the [chan, q] layout the projection consumes.
                nqt = (qn + 127) // 128
                qb = qo // 128
                rc = np_.tile([128, 2, 4], F32, tag="rs", name="rc")
                nc.vector.reciprocal(
                    rc[: min(qn, 128), :, :nqt],
                    po[: min(qn, 128), :, :nqt, HD : HD + 1],
                )
                for h in range(2):
                    for qt in range(nqt):
                        qw = min(128, qn - 128 * qt)
                        nc.vector.tensor_scalar(
                            outq[h][:qw, qb + qt, :],
                            po[:qw, h, qt, 0:HD],
                            rc[:qw, h, qt : qt + 1],
                            None,
                            MULT,
                        )
                for h in range(2):
                    for qt in range(nqt):
                        qw = min(128, qn - 128 * qt)
                        tp = pa.tile([128, 1024], BF, tag="ps", name="tp")
                        nc.tensor.transpose(
                            tp[:HD, :qw], outq[h][:qw, qb + qt, :], idn[:qw, :qw]
                        )
                        aux_copy(
                            outn[h][:, qo + 128 * qt : qo + 128 * qt + qw],
                            tp[:HD, :qw],
                        )

            def project_chunk(qi, qo, qn):
                # y_partial^T[oc, col] = sum_h wproj[oc, my 64h+..] o_h[.., col]
                # for THIS chunk's columns - spreads the projection across the
                # attention instead of a burst per RS part.  PSUM slab -> SBUF
                # bf16 (copies alternate Act/DVE) -> DMA into the RS staging
                # buffer slices of the rank-quarters this chunk touches.
                k = next(
                    i for i, (_, c0, w) in enumerate(PARTS) if c0 <= qo < c0 + w
                )
                _, c0, w = PARTS[k]
                wq_ = w // 4
                ysb = pj.tile([128, 4, 512], BF, tag="ysb")
                for ob in range(4):
                    py = pa.tile([128, 512], F32, tag="ps")
                    for h in range(2):
                        nc.tensor.matmul(
                            py[:, :qn],
                            wpT[:, h, 128 * ob : 128 * (ob + 1)],
                            outn[h][:, qo : qo + qn],
                            start=(h == 0),
                            stop=(h == 1),
                        )
                    aux_copy(ysb[:, ob, :qn], py[:, :qn])
                # one DMA per rank-quarter intersection, all 4 oc-blocks at
                # once: dst rows 512r+128ob+p reached via a (o p) w -> p o w
                # view of the block
                for r in range(4):
                    lo = max(qo, c0 + r * wq_)
                    hi = min(qo + qn, c0 + (r + 1) * wq_)
                    if hi <= lo:
                        continue
                    dst = ri[k][
                        512 * r : 512 * (r + 1),
                        lo - c0 - r * wq_ : hi - c0 - r * wq_,
                    ].rearrange("(o p) w -> p o w", o=4)
                    nc.sync.dma_start(dst, ysb[:, :, lo - qo : hi - qo])

            def run_rs(k):
                nc.gpsimd.collective_compute(
                    "ReduceScatter",
                    ADD,
                    replica_groups=GROUPS,
                    ins=[ri[k].opt()],
                    outs=[ro[k].opt()],
                )

            def flush_out(k):
                _, _, w = PARTS[k]
                lo = sum(pw // 4 for _, _, pw in PARTS[:k])
                nc.sync.dma_start(out_d[:, lo : lo + w // 4], ro[k][:])

            for qi, (qo, qn) in enumerate(QCH):
                # [q, head, qtile, hd|denom|pad]: each head's four qt regions
                # share one 2KB PSUM bank = one zero region - so only the
                # first matmul starts the group and only the last stops it
                po = pb.tile([128, 2, 4, 128], F32, tag="po")
                if qn == 512:
                    # software-pipelined: av(mi) issues two key-tiles behind
                    # qk(mi) so the in-order PE queue never waits for an exp
                    DEPTH = 2
                    ats = {}
                    for mi in range(len(MT) + DEPTH):
                        if mi < len(MT):
                            mo, mn = MT[mi]
                            pph = [
                                pa.tile([128, 512], F32, tag="ps", name=f"pp{h}")
                                for h in range(2)
                            ]
                            at = ap_.tile([128, 2, 512], BF, tag="at")
                            ats[mi] = at
                            for h in range(2):
                                hs = slice(64 * h, 64 * (h + 1))
                                nc.tensor.matmul(
                                    pph[h][:mn, :qn],
                                    kT[hs, mo : mo + mn],
                                    qT[hs, qo : qo + qn],
                                    start=True,
                                    stop=True,
                                )
                            nc.scalar.activation(
                                at[:mn, 0, :qn], pph[0][:mn, :qn], EXP
                            )
                            fast_exp(at[:mn, 1, :qn], pph[1][:mn, :qn])
                        mj = mi - DEPTH
                        if mj >= 0:
                            pmo, pmn = MT[mj]
                            pat = ats.pop(mj)
                            for h in range(2):
                                for qt in range(4):
                                    nc.tensor.matmul(
                                        po[:, h, qt, 0 : HD + 1],
                                        pat[:pmn, h, 128 * qt : 128 * (qt + 1)],
                                        v1[:pmn, mj, h, :],
                                        start=(mj == 0 and qt == 0),
                                        stop=(mj == len(MT) - 1 and qt == 3),
                                    )
                        if qi == 0 and 2 <= mi + 2 < len(MT):
                            produce_v1(mi + 2)
                        if qi == 0 and mi % 4 == 0 and 2 + mi // 4 < len(QCH):
                            # kT chunk j must exist before mt reaches key 512j
                            produce_chunk(wkT, kT, *QCH[2 + mi // 4])
                        elif 1 <= qi <= 4 and mi in (8, 17):
                            j = 2 + (qi - 1) * 2 + (mi - 8) // 9
                            if j < len(QCH) and j > qi:
                                produce_chunk(wqT, qT, *QCH[j])
                else:
                    # ragged 64-query tail: pack 8 key tiles x 2 heads per
                    # psum tile (8 slots of 64 per bank) so exp stays
                    # amortized at N=1024
                    groups = [
                        list(enumerate(MT))[g0 : g0 + 8]
                        for g0 in range(0, len(MT), 8)
                    ]
                    gats = {}
                    for gi in range(len(groups) + 1):
                        if gi < len(groups):
                            ms = groups[gi]
                            pph = [
                                pa.tile([128, 512], F32, tag="ps", name=f"pp{h}")
                                for h in range(2)
                            ]
                            at = ap_.tile([128, 2, 512], BF, tag="at")
                            gats[gi] = at
                            for s, (mi, (mo, mn)) in enumerate(ms):
                                for h in range(2):
                                    hs = slice(64 * h, 64 * (h + 1))
                                    nc.tensor.matmul(
                                        pph[h][:mn, 64 * s : 64 * s + qn],
                                        kT[hs, mo : mo + mn],
                                        qT[hs, qo : qo + qn],
                                        start=True,
                                        stop=True,
                                    )
                            pmn = max(mn for _, (_, mn) in ms)
                            nc.scalar.activation(
                                at[:pmn, 0, 0 : 64 * len(ms)],
                                pph[0][:pmn, 0 : 64 * len(ms)],
                                EXP,
                            )
                            fast_exp(
                                at[:pmn, 1, 0 : 64 * len(ms)],
                                pph[1][:pmn, 0 : 64 * len(ms)],
                            )
                        if gi > 0:
                            at = gats.pop(gi - 1)
                            for s, (mi, (mo, mn)) in enumerate(groups[gi - 1]):
                                for h in range(2):
                                    nc.tensor.matmul(
                                        po[0:64, h, 0, 0 : HD + 1],
                                        at[:mn, h, 64 * s : 64 * s + qn],
                                        v1[:mn, mi, h, :],
                                        start=(mi == 0),
                                        stop=(mi == len(MT) - 1),
                                    )

                normalize(po, qo, qn)
                project_chunk(qi, qo, qn)

                for k, (after, _, _) in enumerate(PARTS):
                    if qi == after:
                        run_rs(k)
                if qi == 4:
                    flush_out(0)
                if qi == 6:
                    flush_out(1)
            flush_out(2)

    nc.compile()
    nc.m = get_hw_module(nc.m)
    return nc


def _shard(x, wq, wk, wv, wproj):
    x = np.asarray(x, dtype=np.float32)
    wq = np.asarray(wq, dtype=np.float32)
    wk = np.asarray(wk, dtype=np.float32)
    wv = np.asarray(wv, dtype=np.float32)
    wproj = np.asarray(wproj, dtype=np.float32)

    xT = [np.ascontiguousarray(x[b].T).astype(BF16) for b in range(B)]
    wpT = np.ascontiguousarray(wproj.T).astype(BF16)  # [in_chan, out_chan]
    in_maps = []
    for c in range(N_CORES):
        b, hb = c // 4, c % 4
        rows = slice(128 * hb, 128 * (hb + 1))
        in_maps.append(
            {
                "xT": xT[b],
                "wqT": np.ascontiguousarray((wq[rows] * SCALE).T).astype(BF16),
                "wkT": np.ascontiguousarray(wk[rows].T).astype(BF16),
                "wvT": np.ascontiguousarray(wv[rows].T).astype(BF16),
                "wpT": np.ascontiguousarray(wpT[rows]),
            }
        )
    return in_maps


def _unshard(results):
    out = np.empty((B, N, DIM), dtype=np.float32)
    for c in range(N_CORES):
        b, r = c // 4, c % 4
        y = np.asarray(results[c]["out"]).astype(np.float32)
        lo = 0
        for _, c0, w in PARTS:
            wq_ = w // 4
            out[b, c0 + r * wq_ : c0 + (r + 1) * wq_, :] = y[:, lo : lo + wq_].T
            lo += wq_
    return out


def _run(inputs, trace=False):
    from concourse.bass_utils import run_bass_kernel_spmd

    if "nc" not in _CACHE:
        _CACHE["nc"] = _build()
    nc = _CACHE["nc"]
    in_maps = _shard(**inputs)
    res = run_bass_kernel_spmd(
        nc, in_maps, core_ids=list(range(N_CORES)), trace=trace
    )
    return _unshard(res.results), res.exec_time_ns


def kernel(**inputs) -> np.ndarray:
    return _run(inputs, trace=False)[0]



# revision 4
# speedup vs baseline: 1.2809x; 1.2809x over previous
"""Distributed multi-head attention kernel for 8 TRN2 NeuronCores.

Reference computation (per batch b):
    q = x @ wq.T ; k = x @ wk.T ; v = x @ wv.T          (8 heads x 64)
    attn = softmax(q k^T / sqrt(64)) ; o = attn @ v
    y = concat_heads(o) @ wproj.T

Sharding: core c owns batch b = c // 4 and query rows
[784*(c%4), 784*(c%4+1)) for ALL 8 heads.  Each core recomputes the
full k/v projections for its batch (the extra ~2.4 GFLOP is far
cheaper than moving megabytes through the ~15 GB/s collective path,
which dominated the previous ReduceScatter design) and emits a
complete, final [512, 784] output slice.  ZERO collectives.

Engine choreography:
  - PE (tensor): qkv projections, qk (two K=64 heads row-tiled into
    array halves via auto tile_position -> they run concurrently),
    av with V STATIONARY (out[hd, q] += v1aug[keys, hd|1].T @ at),
    which both streams efficiently (N=392 bf16) and lands o in the
    [chan, q] layout the projection wants - no PE transposes at all.
    A 65th ones-column in v accumulates the softmax denominator as
    row 64 of the av output.  A tiny K=1 ones-matmul broadcasts the
    reciprocal row across 64 partitions for the normalize multiply.
  - Scalar (Act) + Vector (DVE): exp of both heads of a key-tile in
    ONE FD=784 instruction (the two qk outputs share one PSUM tile),
    alternated between the engines by a tunable quota; DVE runs the
    Schraudolph fast-exp (int16 bits of x*184.665+16256.5 read as
    bf16 ~= e^x), Act runs native Exp.  PSUM->SBUF copies for the
    k/v/q tiles alternate between the engines.
  - k/v/q production is interleaved into the attention loops (just
    in time, like double buffering) so the PE fills exp-paced slack.

All matmuls bf16 with fp32 PSUM accumulation.
"""

import sys

sys.path.insert(0, "/opt/trn_rl_repo")

import ml_dtypes
import numpy as np

B = 2
N = 3136
DIM = 512
HEADS = 8
HD = 64
SCALE = HD**-0.5
N_CORES = 8

QPC = N // 4  # queries per core
CH = 392  # query chunk (2 chunks per core)
NKT = 25  # key tiles: 24x128 + 1x64
MT = [(128 * k, min(128, N - 128 * k)) for k in range(NKT)]
KCH = [(o, min(512, N - o)) for o in range(0, N, 512)]  # kT prod chunks
DEPTH = 3  # av trails qk by DEPTH key-tiles

# Schraudolph fast-exp constants (int16 view of bf16)
EXP_A = 184.66497
EXP_B = 16256.5

# fraction of exp instructions issued to the scalar engine (Act);
# the rest go to DVE.  Act is a bit faster per element but DVE also
# carries reciprocal + normalize work.
ACT_FRAC = 0.62
# fraction of psum->sbuf copies on Act
CPA_FRAC = 0.50

BF16 = ml_dtypes.bfloat16

_CACHE = {}


def _build():
    import concourse.bacc as bacc
    import concourse.mybir as mybir
    import concourse.tile as tile
    from concourse.bass_interp import get_hw_module

    F32 = mybir.dt.float32
    BF = mybir.dt.bfloat16
    I16 = mybir.dt.int16

    nc = bacc.Bacc("TRN2", target_bir_lowering=False, debug=False, num_devices=N_CORES)

    xT_d = nc.dram_tensor("xT", [DIM, N], BF, kind="ExternalInput")
    xqT_d = nc.dram_tensor("xqT", [DIM, QPC], BF, kind="ExternalInput")
    wqT_d = nc.dram_tensor("wqT", [DIM, DIM], BF, kind="ExternalInput")
    wkT_d = nc.dram_tensor("wkT", [DIM, DIM], BF, kind="ExternalInput")
    wvT_d = nc.dram_tensor("wvT", [DIM, DIM], BF, kind="ExternalInput")
    wpT_d = nc.dram_tensor("wpT", [HD, HEADS * 4 * 128], BF, kind="ExternalInput")
    out_d = nc.dram_tensor("out", [DIM, QPC], BF, kind="ExternalOutput")

    EXP = mybir.ActivationFunctionType.Exp
    COPY = mybir.ActivationFunctionType.Copy
    MULT = mybir.AluOpType.mult
    ADD = mybir.AluOpType.add

    with tile.TileContext(nc) as tc:
        with (
            tc.tile_pool(name="const", bufs=1) as cp,
            tc.tile_pool(name="big", bufs=1) as bp,
            tc.tile_pool(name="attn", bufs=DEPTH + 2) as atp,
            tc.tile_pool(name="norm", bufs=2) as rcp,
            tc.tile_pool(name="psum", bufs=2, space="PSUM") as psp,
        ):
            # ---- activation-table preload: a tiny exp issued first so the
            # ~2.7us ACT_TABLE_LOAD overlaps the input DMAs ----
            warm = cp.tile([1, 16], F32)
            nc.vector.memset(warm[:], 0.0)
            warm2 = cp.tile([1, 16], BF)
            nc.scalar.activation(warm2[:], warm[:], EXP)

            # ---- weights + inputs ----
            wqT = cp.tile([128, 4, DIM], BF)
            wkT = cp.tile([128, 4, DIM], BF)
            wvT = cp.tile([128, 4, DIM], BF)
            wpT = cp.tile([HD, HEADS, 4, 128], BF)

            for t, d in ((wkT, wkT_d), (wqT, wqT_d), (wvT, wvT_d)):
                for k in range(4):
                    nc.gpsimd.dma_start(t[:, k, :], d[128 * k : 128 * (k + 1), :])
            nc.gpsimd.dma_start(
                wpT[:].rearrange("p h s c -> p (h s c)"), wpT_d[0:HD, :]
            )

            xT = bp.tile([128, 4, N], BF)
            xqT = bp.tile([128, 4, QPC], BF)
            for k in range(4):
                nc.sync.dma_start(
                    xT[:, k, 0:512], xT_d[128 * k : 128 * (k + 1), 0:512]
                )
            for k in range(4):
                nc.sync.dma_start(xqT[:, k, :], xqT_d[128 * k : 128 * (k + 1), :])
            for lo, hi in ((512, 1792), (1792, N)):
                for k in range(4):
                    nc.sync.dma_start(
                        xT[:, k, lo:hi], xT_d[128 * k : 128 * (k + 1), lo:hi]
                    )

            kT = bp.tile([128, 4, N], BF)
            qT = bp.tile([128, 4, QPC], BF)
            # v1[key, kt, head, hd|1]: 65th column holds ones -> the av
            # matmul accumulates the softmax denominator in out row 64
            v1 = bp.tile([128, NKT, HEADS, HD + 1], BF)
            nc.vector.memset(v1[:, :, :, HD : HD + 1], 1.0)
            outn = bp.tile([HD, HEADS, 2, CH], BF)
            y = bp.tile([128, 4, QPC], BF)

            # ---- engine-alternation helpers ----
            exp_acc = [0.0]

            def exp_use_act():
                exp_acc[0] += ACT_FRAC
                if exp_acc[0] >= 1.0:
                    exp_acc[0] -= 1.0
                    return True
                return False

            cp_acc = [0.0]

            def aux_copy(dst, src):
                cp_acc[0] += CPA_FRAC
                if cp_acc[0] >= 1.0:
                    cp_acc[0] -= 1.0
                    nc.scalar.activation(dst, src, COPY)
                else:
                    nc.vector.tensor_copy(dst, src)

            # ---- production units (interleaved with attention) ----
            def prod_k(s, ci):
                o, n = KCH[ci]
                pp = psp.tile([128, 512], F32, tag="pp", name="pp")
                for k in range(4):
                    nc.tensor.matmul(
                        pp[:, :n],
                        wkT[:, k, 128 * s : 128 * (s + 1)],
                        xT[:, k, o : o + n],
                        start=(k == 0),
                        stop=(k == 3),
                    )
                aux_copy(kT[:, s, o : o + n], pp[:, :n])

            def prod_v(kt):
                mo, mn = MT[kt]
                pp = psp.tile([128, 512], F32, tag="pp", name="pp")
                for k in range(4):
                    nc.tensor.matmul(
                        pp[:mn, :],
                        xT[:, k, mo : mo + mn],
                        wvT[:, k, :],
                        start=(k == 0),
                        stop=(k == 3),
                    )
                aux_copy(
                    v1[:mn, kt, :, 0:HD],
                    pp[:mn, :].rearrange("p (h c) -> p h c", h=HEADS),
                )

            def prod_q(s, ch):
                pp = psp.tile([128, 512], F32, tag="pp", name="pp")
                for k in range(4):
                    nc.tensor.matmul(
                        pp[:, :CH],
                        wqT[:, k, 128 * s : 128 * (s + 1)],
                        xqT[:, k, ch * CH : (ch + 1) * CH],
                        start=(k == 0),
                        stop=(k == 3),
                    )
                aux_copy(qT[:, s, ch * CH : (ch + 1) * CH], pp[:, :CH])

            # production schedule: (ch, hp, slot) -> list of closures
            sched = {}

            def put(ch, hp, slot, fn):
                sched.setdefault((ch, hp, slot), []).append(fn)

            for kt in range(22):  # v tiles 3..24 during (0,0)
                put(0, 0, kt, (lambda t: lambda: prod_v(t))(kt + 3))
            for hp in range(4):
                for c in range(1, 7):  # kT chunks 1..6 of slab hp
                    put(0, hp, 4 * c - 3, (lambda s, c_: lambda: prod_k(s, c_))(hp, c))
                if hp < 3:  # next slab's chunk 0 + next q slab
                    put(0, hp, 22, (lambda s: lambda: prod_k(s, 0))(hp + 1))
                    put(0, hp, 23, (lambda s: lambda: prod_q(s, 0))(hp + 1))
            for s in range(4):  # chunk-1 q slabs during (0,3)
                put(0, 3, 3 + 5 * s, (lambda s_: lambda: prod_q(s_, 1))(s))

            # ---- prologue production: just enough to start (0,0) ----
            prod_k(0, 0)
            prod_q(0, 0)
            for kt in range(3):
                prod_v(kt)

            # ---- attention ----
            for ch in range(2):
                for hp in range(4):
                    po = psp.tile([128, 2, 512], F32, tag="po", name="po", bufs=1)
                    ats = {}
                    for slot in range(NKT + DEPTH):
                        if slot < NKT:
                            mo, mn = MT[slot]
                            ps = psp.tile([128, 2, 512], F32, tag="ps", name="ps")
                            for h2 in range(2):
                                hs = slice(64 * h2, 64 * (h2 + 1))
                                nc.tensor.matmul(
                                    ps[:mn, h2, :CH],
                                    kT[hs, hp, mo : mo + mn],
                                    qT[hs, hp, ch * CH : (ch + 1) * CH],
                                    start=True,
                                    stop=True,
                                )
                            at = atp.tile([128, 2, CH], BF, tag="at", name="at")
                            ats[slot] = at
                            if exp_use_act():
                                nc.scalar.activation(
                                    at[:mn, :, :], ps[:mn, :, :CH], EXP
                                )
                            else:
                                nc.vector.tensor_scalar(
                                    at[:mn, :, :].bitcast(I16),
                                    ps[:mn, :, :CH],
                                    EXP_A,
                                    EXP_B,
                                    MULT,
                                    ADD,
                                )
                        kj = slot - DEPTH
                        if kj >= 0:
                            pmo, pmn = MT[kj]
                            pat = ats.pop(kj)
                            for h2 in range(2):
                                nc.tensor.matmul(
                                    po[0 : HD + 1, h2, :CH],
                                    v1[:pmn, kj, 2 * hp + h2, :],
                                    pat[:pmn, h2, :],
                                    start=(kj == 0),
                                    stop=(kj == NKT - 1),
                                )
                        for fn in sched.get((ch, hp, slot), ()):
                            fn()

                    # normalize: d sits in po row 64; reciprocal -> DMA
                    # partition-broadcast across 64 partitions -> multiply
                    # (the TT may have at most one PSUM operand, so the
                    # broadcast lands in SBUF)
                    rc = rcp.tile([1, 2, CH], F32, tag="rc", name="rc")
                    nc.vector.reciprocal(rc[0:1, :, :], po[64:65, :, :CH])
                    bcs = rcp.tile([HD, 2, CH], F32, tag="bc", name="bcs")
                    nc.gpsimd.partition_broadcast(
                        bcs[0:HD, :, :], rc[0:1, :, :], channels=HD
                    )
                    nc.vector.tensor_tensor(
                        outn[0:HD, 2 * hp : 2 * hp + 2, ch, :],
                        po[0:HD, :, :CH],
                        bcs[0:HD, :, :],
                        MULT,
                    )

                # ---- output projection for this chunk ----
                for s in range(4):
                    py = psp.tile([128, 512], F32, tag="pp", name="py")
                    for h in range(HEADS):
                        nc.tensor.matmul(
                            py[:, :CH],
                            wpT[:, h, s, :],
                            outn[0:HD, h, ch, :],
                            start=(h == 0),
                            stop=(h == HEADS - 1),
                        )
                    aux_copy(y[:, s, ch * CH : (ch + 1) * CH], py[:, :CH])
                    nc.sync.dma_start(
                        out_d[128 * s : 128 * (s + 1), ch * CH : (ch + 1) * CH],
                        y[:, s, ch * CH : (ch + 1) * CH],
                    )

    nc.compile()
    nc.m = get_hw_module(nc.m)
    return nc


def _shard(x, wq, wk, wv, wproj):
    x = np.asarray(x, dtype=np.float32)
    wq = np.asarray(wq, dtype=np.float32)
    wk = np.asarray(wk, dtype=np.float32)
    wv = np.asarray(wv, dtype=np.float32)
    wproj = np.asarray(wproj, dtype=np.float32)

    xT = [np.ascontiguousarray(x[b].T).astype(BF16) for b in range(B)]
    wqT = np.ascontiguousarray((wq * SCALE).T).astype(BF16)
    wkT = np.ascontiguousarray(wk.T).astype(BF16)
    wvT = np.ascontiguousarray(wv.T).astype(BF16)
    wpT = np.empty((HD, HEADS, 4, 128), dtype=np.float32)
    for h in range(HEADS):
        for s in range(4):
            wpT[:, h, s, :] = wproj[128 * s : 128 * (s + 1), 64 * h : 64 * (h + 1)].T
    wpT = np.ascontiguousarray(wpT.reshape(HD, HEADS * 4 * 128)).astype(BF16)

    in_maps = []
    for c in range(N_CORES):
        b, r = c // 4, c % 4
        in_maps.append(
            {
                "xT": xT[b],
                "xqT": np.ascontiguousarray(xT[b][:, QPC * r : QPC * (r + 1)]),
                "wqT": wqT,
                "wkT": wkT,
                "wvT": wvT,
                "wpT": wpT,
            }
        )
    return in_maps


def _unshard(results):
    out = np.empty((B, N, DIM), dtype=np.float32)
    for c in range(N_CORES):
        b, r = c // 4, c % 4
        y = np.asarray(results[c]["out"]).astype(np.float32)
        out[b, QPC * r : QPC * (r + 1), :] = y.T
    return out


def _run(inputs, trace=False):
    from concourse.bass_utils import run_bass_kernel_spmd

    if "nc" not in _CACHE:
        _CACHE["nc"] = _build()
    nc = _CACHE["nc"]
    in_maps = _shard(**inputs)
    res = run_bass_kernel_spmd(
        nc, in_maps, core_ids=list(range(N_CORES)), trace=trace
    )
    return _unshard(res.results), res.exec_time_ns


def kernel(**inputs) -> np.ndarray:
    return _run(inputs, trace=False)[0]
